# revision 11
# baseline (speedup 1.0000x reference)
"""CE + CJS loss kernel for Trainium2, data-parallel over 8 NeuronCores.

Math (reference):
    logp = log_softmax(pred_logit, axis=1)          # x - lse_i
    ce   = -mean_i( sum_j gt*logp )
    p    = softmax(pred_logit)
    m    = 0.5*(gt + p + EPS)
    contrib = gt*ln(gt) + p*logp - (gt+p)*ln(m)     # per element
    cjs  = 0.5 * sum_ij w_j * contrib_ij / B,  w_j = C - j
    loss = ce + 0.5*cjs

Kernel decomposition (v2):
    With xp = x - lse, u = gt + p, q = xp - logm:
        f1 = gt*lngt, f2 = u*q, f4 = gt*xp
        contrib = f1 + f2 - f4          (exactly)
        CE total = sum_ij f4
    Per-column sums of f1+f2 and of f4 accumulate in two PSUM bank sets
    via ones-vector matmuls; host applies the w_j weighting and the
    subtraction in float64.
    Engine balance per core (~89us HBM roofline):
      ScalarE: Exp(x)+rowsum, Ln(gt), Ln(m)            (3 passes)
      VectorE: p, u, xp, f1, q, f2                     (~2.5 cyc/elem)
      GpSimd:  f4 = gt*xp + the casting input DMAs
      TensorE: 3 colsum streams (24 matmuls / 4096-chunk)
    Inputs are loaded as bf16 via gpsimd casting DMAs (f32 in HBM).
"""
import numpy as np

import concourse.bass as bass
import concourse.tile as tile
from concourse import mybir
from concourse.bass_utils import run_bass_kernel_spmd
from concourse.vector_clock import ScopedClock

B, C = 4096, 8192
N_CORES = 8
ROWS = B // N_CORES          # 512 rows per core
N_BLK = ROWS // 128          # 4 partition blocks
F2 = 4096                    # chunk width
N_CHUNK = C // F2            # 2 chunks per block
NSL = F2 // 512              # 8 matmul slices per chunk
N_SLICE = C // 512           # 16 column slices total
EPS = 1e-8

# config flags (fallbacks for primitives that may not work on HW)
X_CAST_DMA = True    # x via gpsimd casting DMA (f32 HBM -> bf16 SBUF)
GT_CAST_DMA = True   # gt likewise
G_F4 = 8             # how many of the 8 chunks compute f4 on GpSimd
MM96 = True          # 4-slices-per-PSUM-bank packing (needs col base 96)

f32 = mybir.dt.float32
bf16 = mybir.dt.bfloat16
AF = mybir.ActivationFunctionType
ALU = mybir.AluOpType


def _patched_drain_and_barrier(self, tick_clock, wait_clock):
    # Walrus CoreV3 codegen allows only ONE sync-wait command on a
    # Drain/NoOp (NO_STRUCT ctrl). The stock Tile tail drain carries one
    # wait per pending engine clock and fails to compile. Split the waits
    # across single-wait SP nops; SP executes in program order, so the
    # drain still orders after everything.
    nc = self.nc
    probe = nc.sync.nop().ins
    wait_clock.add_sem_waits(probe, ScopedClock({None: tick_clock.global_clock}))
    waits = list(probe.sync_info.on_wait) if probe.sync_info else []
    probe.sync_info = mybir.SyncInfo(on_wait=waits[:1], on_update=[])
    for w in waits[1:]:
        extra = nc.sync.nop().ins
        extra.sync_info = mybir.SyncInfo(on_wait=[w], on_update=[])
    nc.sync.drain()
    nc.all_engine_barrier()
    assert self.sems is not None
    popped = nc._tile_sem_poison_stack.pop()
    assert popped is self._sem_poison
    nc.clear_and_free_semaphores(list(self.sems.allocated().values()))
    nc.all_engine_barrier()


tile.TileContext._drain_and_barrier = _patched_drain_and_barrier


def _split_excess_waits(nc: bass.Bass, max_waits: int = 1):
    # Same walrus limitation, general form: cap sync waits per instruction,
    # hoisting the excess onto same-engine NOPs inserted just before (the
    # engine executes its stream in order, so semantics are unchanged).
    for bb in nc.main_func.blocks:
        insts = list(bb.instructions)
        out, changed = [], False
        for ins in insts:
            si = ins.sync_info
            waits = list(si.on_wait) if (si is not None and si.on_wait) else []
            if len(waits) > max_waits:
                ups = list(si.on_update) if si.on_update else []
                for w in waits[:-max_waits]:
                    nop = mybir.InstNoOp(
                        name=nc.get_next_instruction_name(), ins=[], outs=[])
                    nop.engine = ins.engine
                    nop.sync_info = mybir.SyncInfo(on_wait=[w], on_update=[])
                    nc.register_instruction(nop)
                    out.append(nop)
                ins.sync_info = mybir.SyncInfo(
                    on_wait=waits[-max_waits:], on_update=ups)
                changed = True
            out.append(ins)
        if changed:
            bb.instructions = out


def build_nc() -> bass.Bass:
    nc = bass.Bass()
    x_dram = nc.declare_dram_parameter("pred_logit", [ROWS, C], f32, isOutput=False)
    gt_dram = nc.declare_dram_parameter("gt", [ROWS, C], f32, isOutput=False)
    cs_dram = nc.declare_dram_parameter("partials", [N_SLICE, 512], f32, isOutput=True)
    f4_dram = nc.declare_dram_parameter("partials_f4", [N_SLICE, 512], f32, isOutput=True)

    from contextlib import ExitStack
    with tile.TileContext(nc) as tc, ExitStack() as es:
        consts = es.enter_context(tc.tile_pool(name="consts", bufs=1))
        xpool = es.enter_context(tc.tile_pool(name="xpool", bufs=2))
        tpool = es.enter_context(tc.tile_pool(name="tpool", bufs=2))
        rowp = es.enter_context(tc.tile_pool(name="rowp", bufs=2))
        gtp = es.enter_context(tc.tile_pool(name="gtp", bufs=4))
        ck = es.enter_context(tc.tile_pool(name="ck", bufs=2))
        psum = es.enter_context(tc.tile_pool(name="psum", bufs=1, space="PSUM"))

        ones = consts.tile([128, 1], bf16)
        nc.vector.memset(ones, 1.0)
        neg_ones = consts.tile([128, 1], bf16)
        nc.vector.memset(neg_ones, -1.0)
        eps_half = consts.tile([128, 1], f32)
        nc.vector.memset(eps_half, 0.5 * EPS)

        if MM96:
            csb = [psum.tile([128, 512], f32, name=f"cs{i}", tag=f"cs{i}")
                   for i in range(4)]
            f4b = [psum.tile([128, 512], f32, name=f"f4{i}", tag=f"f4{i}")
                   for i in range(4)]

            def cs_mm(m, rhs, start, stop):
                base = 32 * (m % 4)
                nc.tensor.matmul(csb[m // 4][base:base + 1, :], ones[:], rhs,
                                 start=start, stop=stop, tile_position=(0, base))

            def f4_mm(m, rhs, start, stop):
                base = 32 * (m % 4)
                nc.tensor.matmul(f4b[m // 4][base:base + 1, :], ones[:], rhs,
                                 start=start, stop=stop, tile_position=(0, base))
        else:
            # fallback: 3 slices/bank at bases 0/32/64 (6 banks) for
            # f1+f2-f4, plus one superimposed CE row; host gets the
            # f4 columns only via the negated cs contribution.
            csb = [psum.tile([128, 512], f32, name=f"cs{i}", tag=f"cs{i}")
                   for i in range(6)]
            ce_psum = psum.tile([1, 512], f32)

            def cs_mm(m, rhs, start, stop):
                base = 32 * (m % 3)
                nc.tensor.matmul(csb[m // 3][base:base + 1, :], ones[:], rhs,
                                 start=start, stop=stop, tile_position=(0, base))

            def f4neg_mm(m, rhs, start, stop):
                base = 32 * (m % 3)
                nc.tensor.matmul(csb[m // 3][base:base + 1, :], neg_ones[:], rhs,
                                 start=start, stop=stop, tile_position=(0, base))

        # lookahead-allocated input tiles + their DMAs (gpsimd queue for
        # casting DMAs must be primed a block ahead of the f4 work)
        xdt = bf16 if X_CAST_DMA else f32
        gdt = bf16 if GT_CAST_DMA else f32
        xtiles, gtiles = {}, {}

        def emit_input_dmas(b):
            r0 = b * 128
            xb = xpool.tile([128, C], xdt, tag="x")
            xtiles[b] = xb
            for h in range(2):
                sl = slice(h * F2, (h + 1) * F2)
                if X_CAST_DMA:
                    nc.gpsimd.dma_start(out=xb[:, sl], in_=x_dram[r0:r0 + 128, sl])
                else:
                    nc.sync.dma_start(out=xb[:, sl], in_=x_dram[r0:r0 + 128, sl])
            for c in range(N_CHUNK):
                sl = slice(c * F2, (c + 1) * F2)
                g = gtp.tile([128, F2], gdt, tag="gt")
                gtiles[(b, c)] = g
                if GT_CAST_DMA:
                    nc.gpsimd.dma_start(out=g[:], in_=gt_dram[r0:r0 + 128, sl])
                else:
                    nc.sync.dma_start(out=g[:], in_=gt_dram[r0:r0 + 128, sl])

        emit_input_dmas(0)
        for b in range(N_BLK):
            xb = xtiles[b]
            tb = tpool.tile([128, C], bf16, tag="t")
            s = rowp.tile([128, 1], f32, tag="s")
            nc.scalar.activation(out=tb[:], in_=xb[:], func=AF.Exp, accum_out=s[:])
            recip = rowp.tile([128, 1], f32, tag="recip")
            nc.vector.reciprocal(out=recip[:], in_=s[:])
            lse = rowp.tile([128, 1], f32, tag="lse")
            nc.scalar.activation(out=lse[:], in_=s[:], func=AF.Ln)

            # prime next block's input DMAs on the gpsimd queue BEFORE
            # this block's f4 ops so the transfers overlap this block
            if b + 1 < N_BLK:
                emit_input_dmas(b + 1)

            # both chunks' Ln(gt) first so ScalarE never stalls behind logm
            lngts = {}
            for c in range(N_CHUNK):
                g = gtiles[(b, c)]
                lngt = ck.tile([128, F2], bf16, tag="lngt")
                nc.scalar.activation(out=lngt[:], in_=g[:], func=AF.Ln)
                lngts[c] = lngt

            for c in range(N_CHUNK):
                sl = slice(c * F2, (c + 1) * F2)
                g = gtiles[(b, c)]
                gt16 = g
                if not GT_CAST_DMA:
                    gt16 = ck.tile([128, F2], bf16, tag="gt16")
                    nc.vector.tensor_copy(out=gt16[:], in_=g[:])

                p = ck.tile([128, F2], bf16, tag="p", bufs=1)
                nc.vector.tensor_scalar(
                    out=p[:], in0=tb[:, sl], scalar1=recip[:], scalar2=None,
                    op0=ALU.mult)
                u = ck.tile([128, F2], bf16, tag="u")
                nc.vector.tensor_tensor(out=u[:], in0=gt16[:], in1=p[:], op=ALU.add)
                xp = ck.tile([128, F2], bf16, tag="xp")
                nc.vector.tensor_scalar(
                    out=xp[:], in0=xb[:, sl], scalar1=lse[:], scalar2=None,
                    op0=ALU.subtract)
                # ScalarE: logm as soon as u lands; VectorE meanwhile does f1
                logm = ck.tile([128, F2], bf16, tag="logm")
                nc.scalar.activation(out=logm[:], in_=u[:], func=AF.Ln,
                                     scale=0.5, bias=eps_half[:])
                f1 = ck.tile([128, F2], bf16, tag="f1", bufs=1)
                nc.vector.tensor_tensor(out=f1[:], in0=gt16[:], in1=lngts[c][:],
                                        op=ALU.mult)
                q = ck.tile([128, F2], bf16, tag="q", bufs=1)
                nc.vector.tensor_tensor(out=q[:], in0=xp[:], in1=logm[:],
                                        op=ALU.subtract)
                f2 = ck.tile([128, F2], bf16, tag="f2", bufs=1)
                nc.vector.tensor_tensor(out=f2[:], in0=u[:], in1=q[:], op=ALU.mult)
                f4 = ck.tile([128, F2], bf16, tag="f4", bufs=1)
                ci = b * N_CHUNK + c
                # spread G_F4 of the 8 global chunks across GpSimd
                g_set = {round(i * 8 / G_F4) for i in range(G_F4)} if G_F4 > 0 else set()
                eng = nc.gpsimd if ci in g_set or G_F4 >= 8 else nc.vector
                eng.tensor_tensor(out=f4[:], in0=gt16[:], in1=xp[:], op=ALU.mult)

                first, last = (b == 0), (b == N_BLK - 1)
                for k in range(NSL):
                    m = c * NSL + k
                    ksl = slice(k * 512, (k + 1) * 512)
                    cs_mm(m, f1[:, ksl], start=first, stop=False)
                for k in range(NSL):
                    m = c * NSL + k
                    ksl = slice(k * 512, (k + 1) * 512)
                    cs_mm(m, f2[:, ksl], start=False, stop=last and MM96)
                if not MM96:
                    for k in range(NSL):
                        m = c * NSL + k
                        ksl = slice(k * 512, (k + 1) * 512)
                        f4neg_mm(m, f4[:, ksl], start=False, stop=last)
                    nc.tensor.matmul(ce_psum[:], ones[:], f4[:, 0:512],
                                     start=first and c == 0, stop=False)
                    for k in range(1, NSL):
                        ksl = slice(k * 512, (k + 1) * 512)
                        nc.tensor.matmul(
                            ce_psum[:], ones[:], f4[:, ksl],
                            start=False,
                            stop=last and c == N_CHUNK - 1 and k == NSL - 1)
                else:
                    for k in range(NSL):
                        m = c * NSL + k
                        ksl = slice(k * 512, (k + 1) * 512)
                        f4_mm(m, f4[:, ksl], start=first, stop=last)

        # PSUM is not DMA-readable: bounce through SBUF (one reused tag).
        if MM96:
            for i in range(4):
                sb = consts.tile([128, 512], f32, tag="sbounce", bufs=2)
                nc.scalar.copy(out=sb[:], in_=csb[i][:])
                for r in range(4):
                    base = 32 * r
                    m = i * 4 + r
                    nc.sync.dma_start(out=cs_dram[m:m + 1, :],
                                      in_=sb[base:base + 1, :])
            for i in range(4):
                sb = consts.tile([128, 512], f32, tag="sbounce", bufs=2)
                nc.scalar.copy(out=sb[:], in_=f4b[i][:])
                for r in range(4):
                    base = 32 * r
                    m = i * 4 + r
                    nc.sync.dma_start(out=f4_dram[m:m + 1, :],
                                      in_=sb[base:base + 1, :])
        else:
            sb_cs = [consts.tile([128, 512], f32, name=f"sbc{i}", tag=f"sbc{i}")
                     for i in range(6)]
            for i in range(6):
                nc.scalar.copy(out=sb_cs[i][:], in_=csb[i][:])
            sb_ce = consts.tile([1, 512], f32)
            nc.scalar.copy(out=sb_ce[:], in_=ce_psum[:])
            for m in range(N_SLICE):
                base = 32 * (m % 3)
                nc.sync.dma_start(out=cs_dram[m:m + 1, :],
                                  in_=sb_cs[m // 3][base:base + 1, :])
            nc.sync.dma_start(out=f4_dram[0:1, :], in_=sb_ce[:])

    _split_excess_waits(nc)
    return nc


_NC_CACHE = None
LAST_EXEC_NS = None
LAST_TRACE = None


def kernel(pred_logit: np.ndarray, gt: np.ndarray) -> np.ndarray:
    global _NC_CACHE, LAST_EXEC_NS, LAST_TRACE
    if _NC_CACHE is None:
        _NC_CACHE = build_nc()
    nc = _NC_CACHE

    pred_logit = np.ascontiguousarray(pred_logit, dtype=np.float32)
    gt = np.ascontiguousarray(gt, dtype=np.float32)
    in_maps = [
        {
            "pred_logit": pred_logit[c * ROWS:(c + 1) * ROWS],
            "gt": gt[c * ROWS:(c + 1) * ROWS],
        }
        for c in range(N_CORES)
    ]
    res = run_bass_kernel_spmd(nc, in_maps, list(range(N_CORES)))
    if res.exec_time_ns is not None:
        LAST_EXEC_NS = res.exec_time_ns
        if res.instructions_and_trace:
            LAST_TRACE = res.instructions_and_trace[1]

    w = (C - np.arange(C)).astype(np.float64)
    e1_total = 0.0   # sum_j w_j * colsum(contrib)_j
    ce_total = 0.0   # sum_ij gt*xp
    for r in res.results:
        cs = r["partials"].astype(np.float64).reshape(C)
        if MM96:
            f4cs = r["partials_f4"].astype(np.float64).reshape(C)
            e1_total += np.dot(w, cs - f4cs)
            ce_total += f4cs.sum()
        else:
            e1_total += np.dot(w, cs)   # f4 already negated into cs
            ce_total += float(r["partials_f4"][0].astype(np.float64).sum())
    loss = -ce_total / B + 0.25 * e1_total / B
    return np.array(loss, dtype=np.float32)


# revision 21
# speedup vs baseline: 1.3618x; 1.3618x over previous
"""CE + CJS loss kernel for Trainium2, data-parallel over 8 NeuronCores.

Math (reference):
    logp = log_softmax(pred_logit, axis=1)          # x - lse_i
    ce   = -mean_i( sum_j gt*logp )
    p    = softmax(pred_logit)
    m    = 0.5*(gt + p + EPS)
    contrib = gt*ln(gt) + p*logp - (gt+p)*ln(m)     # per element
    cjs  = 0.5 * sum_ij w_j * contrib_ij / B,  w_j = C - j
    loss = ce + 0.5*cjs

Kernel decomposition (v2):
    With xp = x - lse, u = gt + p, q = xp - logm:
        f1 = gt*lngt, f2 = u*q, f4 = gt*xp
        contrib = f1 + f2 - f4          (exactly)
        CE total = sum_ij f4
    Per-column sums of f1+f2 and of f4 accumulate in two PSUM bank sets
    via ones-vector matmuls; host applies the w_j weighting and the
    subtraction in float64.
    Engine balance per core (~89us HBM roofline):
      ScalarE: Exp(x)+rowsum, Ln(gt), Ln(m)            (3 passes)
      VectorE: p, u, xp, f1, q, f2                     (~2.5 cyc/elem)
      GpSimd:  f4 = gt*xp + the casting input DMAs
      TensorE: 3 colsum streams (24 matmuls / 4096-chunk)
    Inputs are loaded as bf16 via gpsimd casting DMAs (f32 in HBM).
"""
import numpy as np

import concourse.bass as bass
import concourse.tile as tile
from concourse import mybir
from concourse.bass_utils import run_bass_kernel_spmd
from concourse.vector_clock import ScopedClock

B, C = 4096, 8192
N_CORES = 8
ROWS = B // N_CORES          # 512 rows per core
N_BLK = ROWS // 128          # 4 partition blocks
F2 = 4096                    # chunk width
N_CHUNK = C // F2            # 2 chunks per block
NSL = F2 // 512              # 8 matmul slices per chunk
N_SLICE = C // 512           # 16 column slices total
EPS = 1e-8

# config flags (fallbacks for primitives that may not work on HW)
X_CAST_DMA = True    # x via gpsimd casting DMA (f32 HBM -> bf16 SBUF)
GT_CAST_DMA = True   # gt likewise
G_F4 = 0             # chunks computing f4 on GpSimd (HW: SBUF-port contention
                     # with VectorE makes GpSimd elementwise a net loss)
WARM_MM = True       # data-anchored dummy matmuls to hold the PE HAM warm

f32 = mybir.dt.float32
bf16 = mybir.dt.bfloat16
AF = mybir.ActivationFunctionType
ALU = mybir.AluOpType


def _patched_drain_and_barrier(self, tick_clock, wait_clock):
    # Walrus CoreV3 codegen allows only ONE sync-wait command on a
    # Drain/NoOp (NO_STRUCT ctrl). The stock Tile tail drain carries one
    # wait per pending engine clock and fails to compile. Split the waits
    # across single-wait SP nops; SP executes in program order, so the
    # drain still orders after everything.
    nc = self.nc
    probe = nc.sync.nop().ins
    wait_clock.add_sem_waits(probe, ScopedClock({None: tick_clock.global_clock}))
    waits = list(probe.sync_info.on_wait) if probe.sync_info else []
    probe.sync_info = mybir.SyncInfo(on_wait=waits[:1], on_update=[])
    for w in waits[1:]:
        extra = nc.sync.nop().ins
        extra.sync_info = mybir.SyncInfo(on_wait=[w], on_update=[])
    nc.sync.drain()
    nc.all_engine_barrier()
    assert self.sems is not None
    popped = nc._tile_sem_poison_stack.pop()
    assert popped is self._sem_poison
    nc.clear_and_free_semaphores(list(self.sems.allocated().values()))
    nc.all_engine_barrier()


tile.TileContext._drain_and_barrier = _patched_drain_and_barrier


def _split_excess_waits(nc: bass.Bass, max_waits: int = 1):
    # Same walrus limitation, general form: cap sync waits per instruction,
    # hoisting the excess onto same-engine NOPs inserted just before (the
    # engine executes its stream in order, so semantics are unchanged).
    for bb in nc.main_func.blocks:
        insts = list(bb.instructions)
        out, changed = [], False
        for ins in insts:
            si = ins.sync_info
            waits = list(si.on_wait) if (si is not None and si.on_wait) else []
            if len(waits) > max_waits:
                ups = list(si.on_update) if si.on_update else []
                for w in waits[:-max_waits]:
                    nop = mybir.InstNoOp(
                        name=nc.get_next_instruction_name(), ins=[], outs=[])
                    nop.engine = ins.engine
                    nop.sync_info = mybir.SyncInfo(on_wait=[w], on_update=[])
                    nc.register_instruction(nop)
                    out.append(nop)
                ins.sync_info = mybir.SyncInfo(
                    on_wait=waits[-max_waits:], on_update=ups)
                changed = True
            out.append(ins)
        if changed:
            bb.instructions = out


def build_nc() -> bass.Bass:
    nc = bass.Bass()
    x_dram = nc.declare_dram_parameter("pred_logit", [ROWS, C], f32, isOutput=False)
    gt_dram = nc.declare_dram_parameter("gt", [ROWS, C], f32, isOutput=False)
    cs_dram = nc.declare_dram_parameter("partials", [N_SLICE, 512], f32, isOutput=True)
    f4_dram = nc.declare_dram_parameter("partials_f4", [1, 512], f32, isOutput=True)

    from contextlib import ExitStack
    with tile.TileContext(nc) as tc, ExitStack() as es:
        consts = es.enter_context(tc.tile_pool(name="consts", bufs=1))
        xpool = es.enter_context(tc.tile_pool(name="xpool", bufs=2))
        tpool = es.enter_context(tc.tile_pool(name="tpool", bufs=2))
        rowp = es.enter_context(tc.tile_pool(name="rowp", bufs=2))
        gtp = es.enter_context(tc.tile_pool(name="gtp", bufs=4))
        ck = es.enter_context(tc.tile_pool(name="ck", bufs=2))
        psum = es.enter_context(tc.tile_pool(name="psum", bufs=1, space="PSUM"))

        ones = consts.tile([128, 1], bf16)
        nc.vector.memset(ones, 1.0)
        neg_ones = consts.tile([128, 1], bf16)
        nc.vector.memset(neg_ones, -1.0)
        eps_half = consts.tile([128, 1], f32)
        nc.vector.memset(eps_half, 0.5 * EPS)

        # PSUM: cs = 4 banks x 4 base-partitions = 16 column-slice regions
        # accumulating f1 + f2 - f4; ce = one superimposed [1,512] row of
        # +f4; 3 banks left as scratch for PE-warming dummy matmuls.
        csb = [psum.tile([128, 512], f32, name=f"cs{i}", tag=f"cs{i}")
               for i in range(4)]
        ce_psum = psum.tile([1, 512], f32)
        scratch = [psum.tile([128, 512], f32, name=f"scr{i}", tag=f"scr{i}")
                   for i in range(2)]

        def cs_mm(m, rhs, start, stop, neg=False):
            base = 32 * (m % 4)
            nc.tensor.matmul(csb[m // 4][base:base + 1, :],
                             neg_ones[:] if neg else ones[:], rhs,
                             start=start, stop=stop, tile_position=(0, base))

        _warm_i = [0]

        def warm_mm(anchor_ap):
            # tiny throwaway matmul reading an in-flight tile: executes as
            # soon as its input lands, touching the PE often enough that
            # the HAM clock gate stays at 8/8 between real MM bursts.
            if not WARM_MM:
                return
            i = _warm_i[0] = _warm_i[0] + 1
            scr = scratch[i % 2]
            nc.tensor.matmul(scr[0:1, :128], ones[:], anchor_ap,
                             start=True, stop=True, tile_position=(0, 0))

        # lookahead-allocated input tiles + their DMAs (gpsimd queue for
        # casting DMAs must be primed a block ahead of the f4 work)
        xdt = bf16 if X_CAST_DMA else f32
        gdt = bf16 if GT_CAST_DMA else f32
        xtiles, gtiles = {}, {}

        def emit_input_dmas(b):
            r0 = b * 128
            xb = xpool.tile([128, C], xdt, tag="x")
            xtiles[b] = xb
            for h in range(2):
                sl = slice(h * F2, (h + 1) * F2)
                if X_CAST_DMA:
                    nc.gpsimd.dma_start(out=xb[:, sl], in_=x_dram[r0:r0 + 128, sl])
                else:
                    nc.sync.dma_start(out=xb[:, sl], in_=x_dram[r0:r0 + 128, sl])
            for c in range(N_CHUNK):
                sl = slice(c * F2, (c + 1) * F2)
                g = gtp.tile([128, F2], gdt, tag="gt")
                gtiles[(b, c)] = g
                if GT_CAST_DMA:
                    nc.gpsimd.dma_start(out=g[:], in_=gt_dram[r0:r0 + 128, sl])
                else:
                    nc.sync.dma_start(out=g[:], in_=gt_dram[r0:r0 + 128, sl])

        emit_input_dmas(0)
        for b in range(N_BLK):
            xb = xtiles[b]
            tb = tpool.tile([128, C], bf16, tag="t")
            # exp split per x-DMA half so it starts as soon as half 0 lands
            s2 = rowp.tile([128, 2], f32, tag="s2")
            for h in range(2):
                sl = slice(h * F2, (h + 1) * F2)
                nc.scalar.activation(out=tb[:, sl], in_=xb[:, sl], func=AF.Exp,
                                     accum_out=s2[:, h:h + 1])
                warm_mm(tb[:, h * F2:h * F2 + 128])
            s = rowp.tile([128, 1], f32, tag="s")
            nc.vector.tensor_reduce(out=s[:], in_=s2[:], op=ALU.add,
                                    axis=mybir.AxisListType.X)
            recip = rowp.tile([128, 1], f32, tag="recip")
            nc.vector.reciprocal(out=recip[:], in_=s[:])
            lse = rowp.tile([128, 1], f32, tag="lse")
            nc.scalar.activation(out=lse[:], in_=s[:], func=AF.Ln)

            # prime next block's input DMAs on the gpsimd queue BEFORE
            # this block's f4 ops so the transfers overlap this block
            if b + 1 < N_BLK:
                emit_input_dmas(b + 1)

            # both chunks' Ln(gt) first so ScalarE never stalls behind logm
            lngts = {}
            for c in range(N_CHUNK):
                g = gtiles[(b, c)]
                lngt = ck.tile([128, F2], bf16, tag="lngt")
                nc.scalar.activation(out=lngt[:], in_=g[:], func=AF.Ln)
                lngts[c] = lngt

            for c in range(N_CHUNK):
                sl = slice(c * F2, (c + 1) * F2)
                g = gtiles[(b, c)]
                gt16 = g
                if not GT_CAST_DMA:
                    gt16 = ck.tile([128, F2], bf16, tag="gt16")
                    nc.vector.tensor_copy(out=gt16[:], in_=g[:])

                first, last = (b == 0), (b == N_BLK - 1)
                p = ck.tile([128, F2], bf16, tag="p", bufs=1)
                nc.vector.tensor_scalar(
                    out=p[:], in0=tb[:, sl], scalar1=recip[:], scalar2=None,
                    op0=ALU.mult)
                warm_mm(p[:, 0:128])
                u = ck.tile([128, F2], bf16, tag="u")
                nc.vector.tensor_tensor(out=u[:], in0=gt16[:], in1=p[:], op=ALU.add)
                xp = ck.tile([128, F2], bf16, tag="xp")
                nc.vector.tensor_scalar(
                    out=xp[:], in0=xb[:, sl], scalar1=lse[:], scalar2=None,
                    op0=ALU.subtract)
                warm_mm(xp[:, 0:128])
                # ScalarE: logm as soon as u lands; VectorE meanwhile does f1
                logm = ck.tile([128, F2], bf16, tag="logm")
                nc.scalar.activation(out=logm[:], in_=u[:], func=AF.Ln,
                                     scale=0.5, bias=eps_half[:])
                f1 = ck.tile([128, F2], bf16, tag="f1", bufs=1)
                nc.vector.tensor_tensor(out=f1[:], in0=gt16[:], in1=lngts[c][:],
                                        op=ALU.mult)
                for k in range(NSL):
                    m = c * NSL + k
                    ksl = slice(k * 512, (k + 1) * 512)
                    cs_mm(m, f1[:, ksl], start=first, stop=False)
                q = ck.tile([128, F2], bf16, tag="q", bufs=1)
                nc.vector.tensor_tensor(out=q[:], in0=xp[:], in1=logm[:],
                                        op=ALU.subtract)
                warm_mm(q[:, 0:128])
                f2 = ck.tile([128, F2], bf16, tag="f2", bufs=1)
                nc.vector.tensor_tensor(out=f2[:], in0=u[:], in1=q[:], op=ALU.mult)
                for k in range(NSL):
                    m = c * NSL + k
                    ksl = slice(k * 512, (k + 1) * 512)
                    cs_mm(m, f2[:, ksl], start=False, stop=False)
                f4 = ck.tile([128, F2], bf16, tag="f4", bufs=1)
                ci = b * N_CHUNK + c
                g_set = {round(i * 8 / G_F4) for i in range(G_F4)} if G_F4 > 0 else set()
                eng = nc.gpsimd if ci in g_set or G_F4 >= 8 else nc.vector
                eng.tensor_tensor(out=f4[:], in0=gt16[:], in1=xp[:], op=ALU.mult)
                for k in range(NSL):
                    m = c * NSL + k
                    ksl = slice(k * 512, (k + 1) * 512)
                    cs_mm(m, f4[:, ksl], start=False, stop=last, neg=True)
                for k in range(NSL):
                    ksl = slice(k * 512, (k + 1) * 512)
                    nc.tensor.matmul(
                        ce_psum[:], ones[:], f4[:, ksl],
                        start=first and c == 0 and k == 0,
                        stop=last and c == N_CHUNK - 1 and k == NSL - 1)

        # PSUM is not DMA-readable: bounce through SBUF. Split the bank
        # copies between ScalarE and VectorE to shorten the tail.
        for i in range(4):
            sb = consts.tile([128, 512], f32, tag="sbounce", bufs=2)
            if i % 2 == 0:
                nc.scalar.copy(out=sb[:], in_=csb[i][:])
            else:
                nc.vector.tensor_copy(out=sb[:], in_=csb[i][:])
            for r in range(4):
                base = 32 * r
                m = i * 4 + r
                nc.sync.dma_start(out=cs_dram[m:m + 1, :],
                                  in_=sb[base:base + 1, :])
        sb_ce = consts.tile([1, 512], f32)
        nc.scalar.copy(out=sb_ce[:], in_=ce_psum[:])
        nc.sync.dma_start(out=f4_dram[0:1, :], in_=sb_ce[:])

    _split_excess_waits(nc)
    return nc


_NC_CACHE = None
LAST_EXEC_NS = None
LAST_TRACE = None


def kernel(pred_logit: np.ndarray, gt: np.ndarray) -> np.ndarray:
    global _NC_CACHE, LAST_EXEC_NS, LAST_TRACE
    if _NC_CACHE is None:
        _NC_CACHE = build_nc()
    nc = _NC_CACHE

    pred_logit = np.ascontiguousarray(pred_logit, dtype=np.float32)
    gt = np.ascontiguousarray(gt, dtype=np.float32)
    in_maps = [
        {
            "pred_logit": pred_logit[c * ROWS:(c + 1) * ROWS],
            "gt": gt[c * ROWS:(c + 1) * ROWS],
        }
        for c in range(N_CORES)
    ]
    res = run_bass_kernel_spmd(nc, in_maps, list(range(N_CORES)))
    if res.exec_time_ns is not None:
        LAST_EXEC_NS = res.exec_time_ns
        if res.instructions_and_trace:
            LAST_TRACE = res.instructions_and_trace[1]

    w = (C - np.arange(C)).astype(np.float64)
    e1_total = 0.0   # sum_j w_j * colsum(contrib)_j  (f4 negated on-device)
    ce_total = 0.0   # sum_ij gt*xp
    for r in res.results:
        cs = r["partials"].astype(np.float64).reshape(C)
        e1_total += np.dot(w, cs)
        ce_total += float(r["partials_f4"][0].astype(np.float64).sum())
    loss = -ce_total / B + 0.25 * e1_total / B
    return np.array(loss, dtype=np.float32)
